# revision 1
# baseline (speedup 1.0000x reference)
"""Multi-head attention (B=2, S=2048, D=1024, H=16) on 8 Trainium2 cores.

Sharding: data-parallel over batch (2) x tensor-parallel over heads (16 -> 4
per core). Core c handles batch c//4, heads 4*(c%4) .. 4*(c%4)+3. Each core
computes its heads' Q/K/V projections (column-sliced weights), flash-style
attention with transposed-score layout, and a partial output projection
(row-sliced Wo). Host sums the 4 partials per batch and adds bv@Wo + bo.

Kernel layout notes:
  - All matmuls contract over the partition dim, so x^T ([D, S], d on
    partitions) is built once per core via PE transpose.
  - qT/kT are produced pair-packed [128 = 2 heads x 64, S]; scores^T
    [j, i] come from two row-packed (tile_position) K=64 matmuls sharing
    the PE array.
  - softmax denominators ride along as a 65th row of the AV matmul
    (v augmented with a ones column); normalization is a reciprocal +
    rank-1 ones-outer-product broadcast + DVE multiply into the Wo lhsT.
  - Matmuls run as float32r (FP22 truncated mantissa, full PE rate).
"""

import numpy as np

B, S, D, H, DK = 2, 2048, 1024, 16, 64
HPC = 4          # heads per core
HD = HPC * DK    # 256 projected dims per core
P = 128
NB = 512
NCORES = 8

_CACHE = {}


def _install_tile_drain_fix():
    """TileContext._drain_and_barrier piles every outstanding sem wait onto
    one Drain instruction; this walrus build rejects >1 sync wait per
    instruction. Split the extra waits across single-wait NOPs."""
    import concourse.tile as tile
    from concourse.vector_clock import ScopedClock

    if getattr(tile.TileContext, "_ant_drain_fix", False):
        return

    def _drain_and_barrier_split(self, tick_clock, wait_clock):
        drain_inst = self.nc.sync.drain()
        wait_clock.add_sem_waits(
            drain_inst.ins, ScopedClock({None: tick_clock.global_clock})
        )
        waits = list(drain_inst.ins.sync_info.on_wait or [])
        if len(waits) > 1:
            drain_inst.ins.sync_info.on_wait = waits[:1]
            for w in waits[1:]:
                n = self.nc.sync.nop(nofuse=True)
                si = n.ins.sync_info
                if si is None:
                    import bass_rust

                    n.ins.sync_info = bass_rust.SyncInfo(on_wait=[w], on_update=[])
                else:
                    si.on_wait = [w]

        self.nc.all_engine_barrier()
        assert self.sems is not None
        popped = self.nc._tile_sem_poison_stack.pop()
        assert popped is self._sem_poison
        self.nc.clear_and_free_semaphores(list(self.sems.allocated().values()))
        self.nc.all_engine_barrier()

    tile.TileContext._drain_and_barrier = _drain_and_barrier_split
    tile.TileContext._ant_drain_fix = True


def _split_excess_waits(nc):
    """walrus's per-struct sync-wait capacity is small (observed: 1 for the
    self-loading-weight Matmult S3_LW struct, 2 for TPB_CTRL/Drain). Tile's
    wait assignment can leave many waits on one instruction; hoist the excess
    onto NOPs on the same engine immediately before it."""
    import concourse.mybir as mybir

    nid = [0]
    for f in nc.m.functions:
        for bb in f.blocks:
            out = []
            changed = False
            for inst in bb.instructions:
                si = getattr(inst, "sync_info", None)
                waits = list(si.on_wait) if si is not None and si.on_wait else []
                cap = 1
                if len(waits) > cap:
                    extra = waits[cap:]
                    for k in range(0, len(extra), 2):
                        nid[0] += 1
                        out.append(
                            mybir.InstEventSemaphore(
                                name=f"I-waitsplit-{nid[0]}",
                                ins=[],
                                outs=[],
                                sync_info=mybir.SyncInfo(
                                    on_wait=extra[k:k + 2], on_update=[]
                                ),
                                engine=inst.engine,
                            )
                        )
                    si.on_wait = waits[:cap]
                    changed = True
                out.append(inst)
            if changed:
                bb.instructions = out


def _recip_fast(nc, out, in_):
    with nc.allow_low_precision("fp22 recip feeds f32r matmul"):
        nc.vector.reciprocal(out=out, in_=in_)


def _build_program():
    import concourse.bass as bass
    import concourse.mybir as mybir
    from concourse.masks import make_identity
    from concourse.tile import TileContext

    _install_tile_drain_fix()

    f32 = mybir.dt.float32
    f32r = mybir.dt.float32r
    bf16 = mybir.dt.bfloat16
    Exp = mybir.ActivationFunctionType.Exp

    nc = bass.Bass()

    xb = nc.dram_tensor("xb", [S, D], f32, kind="ExternalInput")
    wq = nc.dram_tensor("wq", [D, HD], f32r, kind="ExternalInput")
    wk = nc.dram_tensor("wk", [D, HD], f32r, kind="ExternalInput")
    wv = nc.dram_tensor("wv", [D, HD], f32r, kind="ExternalInput")
    wo = nc.dram_tensor("wo", [HD, D], f32r, kind="ExternalInput")
    bqt = nc.dram_tensor("bqt", [P, 2], f32, kind="ExternalInput")
    bkt = nc.dram_tensor("bkt", [P, 2], f32, kind="ExternalInput")
    outp = nc.dram_tensor("outp", [S, D], f32, kind="ExternalOutput")

    NDC = D // P      # 8 d-chunks
    NST = S // P      # 16 sequence tiles
    NSB = S // NB     # 4 sequence blocks


    with TileContext(nc) as tc:
        with tc.tile_pool(name="consts", bufs=1) as consts:
            ident = consts.tile([P, P], f32)
            make_identity(nc, ident)
            # memset on a float32r AP emits invalid ISA; write the f32 bit
            # pattern of 1.0 through a uint32 view instead
            onesg = consts.tile([33, DK], f32r)
            nc.vector.memset(onesg.bitcast(mybir.dt.uint32), 0x3F800000)

            wq_sb = consts.tile([P, NDC, HD], f32r)
            nc.sync.dma_start(wq_sb[:], wq.rearrange("(c p) h -> p c h", p=P))
            wk_sb = consts.tile([P, NDC, HD], f32r)
            nc.sync.dma_start(wk_sb[:], wk.rearrange("(c p) h -> p c h", p=P))
            wv_sb = consts.tile([P, NDC, HD], f32r)
            nc.sync.dma_start(wv_sb[:], wv.rearrange("(c p) h -> p c h", p=P))
            wo_sb = consts.tile([P, 2, D], f32r)
            nc.sync.dma_start(wo_sb[:], wo.rearrange("(c p) d -> p c d", p=P))
            bq_sb = consts.tile([P, 2], f32)
            nc.sync.dma_start(bq_sb[:], bqt[:])
            bk_sb = consts.tile([P, 2], f32)
            nc.sync.dma_start(bk_sb[:], bkt[:])

            with tc.tile_pool(name="acts", bufs=1) as acts:
                # pair-packed transposed projections: [2 heads x 64, S]
                qT = acts.tile([P, 2, S], f32r)
                kT = acts.tile([P, 2, S], f32r)
                # v augmented with a ones column (row 65 of the AV matmul
                # accumulates the softmax denominator): [s, j-tile, head, 65]
                va = acts.tile([P, NST, HPC, DK + 1], bf16)
                nc.vector.memset(va.bitcast(mybir.dt.uint16), 0x3F80)

                with (
                    tc.tile_pool(name="ph1", bufs=1) as ph1,
                    tc.tile_pool(name="ph1p", bufs=1, space="PSUM") as ph1p,
                ):
                    xT = ph1.tile([P, NDC, S], f32r)
                    # pipelined: per row-tile do transposes + v projection;
                    # every 4th tile the q/k projections for that s-block
                    for it in range(NST):
                        xr = ph1.tile([P, D], f32, tag="xr", bufs=3, name=f"xr{it}")
                        nc.sync.dma_start(xr[:], xb[it * P:(it + 1) * P, :])
                        for g in range(2):
                            pst = ph1p.tile(
                                [P, 4, P], f32, tag="tr", bufs=3,
                                name=f"tr{it}_{g}",
                            )
                            for dd in range(4):
                                d = g * 4 + dd
                                nc.tensor.transpose(
                                    pst[:, dd, :], xr[:, d * P:(d + 1) * P],
                                    ident[:],
                                )
                            nc.vector.tensor_copy(
                                out=xT[:, g * 4:(g + 1) * 4, it * P:(it + 1) * P],
                                in_=pst[:],
                            )

                        ps = ph1p.tile(
                            [P, HD], f32, tag="vproj", bufs=2, name=f"psv{it}"
                        )
                        for d in range(NDC):
                            nc.tensor.matmul(
                                ps[:],
                                xT[:, d, it * P:(it + 1) * P],
                                wv_sb[:, d, :],
                                start=(d == 0),
                                stop=(d == NDC - 1),
                            )
                        nc.vector.tensor_copy(
                            out=va[:, it, :, 0:DK],
                            in_=ps.rearrange("p (h e) -> p h e", h=HPC),
                        )

                        if it % 4 != 3:
                            continue
                        sb = it // 4
                        for w_sb, b_sb, dT, nm in (
                            (wk_sb, bk_sb, kT, "k"),
                            (wq_sb, bq_sb, qT, "q"),
                        ):
                            for p in range(2):
                                psq = ph1p.tile(
                                    [P, NB], f32, tag="proj", bufs=2,
                                    name=f"ps{nm}{p}_{sb}",
                                )
                                for d in range(NDC):
                                    nc.tensor.matmul(
                                        psq[:],
                                        w_sb[:, d, p * P:(p + 1) * P],
                                        xT[:, d, sb * NB:(sb + 1) * NB],
                                        start=(d == 0),
                                        stop=(d == NDC - 1),
                                    )
                                nc.vector.tensor_scalar_add(
                                    out=dT[:, p, sb * NB:(sb + 1) * NB],
                                    in0=psq[:],
                                    scalar1=b_sb[:, p:p + 1],
                                )

                with (
                    tc.tile_pool(name="ph2", bufs=1) as ph2,
                    tc.tile_pool(name="ph2p", bufs=1, space="PSUM") as ph2p,
                ):
                    # Wo lhsT: [head-dim pair-chunk, pair, i]
                    stack = ph2.tile([P, 2, S], f32r)
                    # staging tile for both heads' softmax denominators, at
                    # partitions 0 and 32 so one reciprocal covers both; the
                    # filler rows are preset to 1.0 so recip never sees junk
                    sums_sb = ph2.tile([33, NB], f32)
                    nc.vector.memset(sums_sb[:], 1.0)

                    def finish_block(ib, p, po_sbs, rc33):
                        # broadcast each head's 1/sumexp across 64 partitions
                        # (rank-1 matmul) and scale the raw AV numerators into
                        # the Wo lhsT. Emitted one block late so the PE never
                        # waits on the DVE reciprocals.
                        i0 = ib * NB
                        for hh in range(2):
                            bc = ph2p.tile(
                                [DK, NB], f32, tag="bc", bufs=1,
                                name=f"bc{ib}_{p}_{hh}",
                            )
                            nc.tensor.matmul(
                                bc[:],
                                onesg[32 * hh:32 * hh + 1, :],
                                rc33[32 * hh:32 * hh + 1, :],
                                tile_position=(32 * hh, 0),
                            )
                            nc.vector.tensor_tensor(
                                out=stack[hh * DK:(hh + 1) * DK, p, i0:i0 + NB],
                                in0=po_sbs[hh][0:DK, :],
                                in1=bc[:],
                                op=mybir.AluOpType.mult,
                            )
                        if p != 1:
                            return
                        # final projection for this i-block (4 row tiles)
                        for t in range(NB // P):
                            it = ib * (NB // P) + t
                            ot = ph2.tile(
                                [P, D], f32, tag="ot", bufs=3, name=f"ot{it}"
                            )
                            for nbi in range(2):
                                ps = ph2p.tile(
                                    [P, NB], f32, tag="fin", bufs=1,
                                    name=f"fin{it}_{nbi}",
                                )
                                for pch in range(2):
                                    nc.tensor.matmul(
                                        ps[:],
                                        stack[:, pch, it * P:(it + 1) * P],
                                        wo_sb[:, pch, nbi * NB:(nbi + 1) * NB],
                                        start=(pch == 0),
                                        stop=(pch == 1),
                                    )
                                nc.vector.tensor_copy(
                                    out=ot[:, nbi * NB:(nbi + 1) * NB], in_=ps[:]
                                )
                            nc.sync.dma_start(
                                outp[it * P:(it + 1) * P, :], ot[:]
                            )

                    pending = None
                    for ib in range(NSB):
                        i0 = ib * NB
                        for p in range(2):
                            po0 = ph2p.tile(
                                [P, NB], f32, tag="av0", bufs=1, name=f"po0_{ib}_{p}"
                            )
                            po1 = ph2p.tile(
                                [P, NB], f32, tag="av1", bufs=1, name=f"po1_{ib}_{p}"
                            )
                            def emit_scores(j):
                                sc = ph2p.tile(
                                    [P, 2 * NB], f32, tag="sc", bufs=2,
                                    name=f"sc{ib}_{p}_{j}",
                                )
                                nc.tensor.matmul(
                                    sc[:, 0:NB],
                                    kT[0:DK, p, j * P:(j + 1) * P],
                                    qT[0:DK, p, i0:i0 + NB],
                                    tile_position=(0, 0),
                                )
                                nc.tensor.matmul(
                                    sc[:, NB:2 * NB],
                                    kT[DK:2 * DK, p, j * P:(j + 1) * P],
                                    qT[DK:2 * DK, p, i0:i0 + NB],
                                    tile_position=(64, 0),
                                )
                                return sc

                            # scores for j+1 are emitted before AV(j) so the
                            # PE FIFO never queues behind exp(j)
                            sc = emit_scores(0)
                            for j in range(NST):
                                ex = ph2.tile(
                                    [P, 2 * NB], bf16, tag="ex", bufs=3,
                                    name=f"ex{ib}_{p}_{j}",
                                )
                                nc.scalar.activation(
                                    ex[:], sc[:], Exp, scale=0.125
                                )
                                if j + 1 < NST:
                                    sc = emit_scores(j + 1)
                                nc.tensor.matmul(
                                    po0[0:DK + 1, :],
                                    va[:, j, 2 * p, :],
                                    ex[:, 0:NB],
                                    start=(j == 0),
                                    stop=(j == NST - 1),
                                )
                                nc.tensor.matmul(
                                    po1[0:DK + 1, :],
                                    va[:, j, 2 * p + 1, :],
                                    ex[:, NB:2 * NB],
                                    start=(j == 0),
                                    stop=(j == NST - 1),
                                )
                            # drain both accumulator banks so the next block's
                            # AV can start, then one batched reciprocal on DVE
                            po_sbs = []
                            for hh, po in ((0, po0), (1, po1)):
                                po_sb = ph2.tile(
                                    [DK, NB], f32, tag="posb", bufs=4,
                                    name=f"posb{ib}_{p}_{hh}",
                                )
                                nc.vector.tensor_copy(
                                    out=po_sb[:], in_=po[0:DK, :]
                                )
                                po_sbs.append(po_sb)
                            nc.vector.tensor_copy(
                                out=sums_sb[0:1, :], in_=po0[DK:DK + 1, :]
                            )
                            nc.vector.tensor_copy(
                                out=sums_sb[32:33, :], in_=po1[DK:DK + 1, :]
                            )
                            rc33 = ph2.tile(
                                [33, NB], f32r, tag="rc", bufs=2,
                                name=f"rc{ib}_{p}",
                            )
                            _recip_fast(nc, rc33[:], sums_sb[:])
                            if pending is not None:
                                finish_block(*pending)
                            pending = (ib, p, po_sbs, rc33)
                    finish_block(*pending)

    _split_excess_waits(nc)
    return nc


def _get_program():
    if "nc" not in _CACHE:
        _CACHE["nc"] = _build_program()
    return _CACHE["nc"]


def kernel(x, Wq, bq, Wk, bk, Wv, bv, Wo, bo, _trace=False):
    from concourse.bass_utils import run_bass_kernel_spmd

    x = np.asarray(x, dtype=np.float32)
    Wq = np.asarray(Wq, dtype=np.float32)
    Wk = np.asarray(Wk, dtype=np.float32)
    Wv = np.asarray(Wv, dtype=np.float32)
    Wo = np.asarray(Wo, dtype=np.float32)
    bq = np.asarray(bq, dtype=np.float32)
    bk = np.asarray(bk, dtype=np.float32)
    bv = np.asarray(bv, dtype=np.float32)
    bo = np.asarray(bo, dtype=np.float32)

    in_maps = []
    for c in range(NCORES):
        b = c // 4
        cs = (c % 4) * HD
        in_maps.append({
            "xb": np.ascontiguousarray(x[b]),
            "wq": np.ascontiguousarray(Wq[:, cs:cs + HD]),
            "wk": np.ascontiguousarray(Wk[:, cs:cs + HD]),
            "wv": np.ascontiguousarray(Wv[:, cs:cs + HD]),
            "wo": np.ascontiguousarray(Wo[cs:cs + HD, :]),
            "bqt": np.ascontiguousarray(bq[cs:cs + HD].reshape(2, P).T),
            "bkt": np.ascontiguousarray(bk[cs:cs + HD].reshape(2, P).T),
        })

    nc = _get_program()
    res = run_bass_kernel_spmd(
        nc, in_maps, core_ids=list(range(NCORES)), trace=_trace
    )

    cvec = (bv @ Wo + bo).astype(np.float32)
    out = np.empty((B, S, D), dtype=np.float32)
    for b in range(B):
        acc = res.results[4 * b]["outp"].astype(np.float64)
        for c in range(4 * b + 1, 4 * b + 4):
            acc = acc + res.results[c]["outp"]
        out[b] = (acc + cvec).astype(np.float32)

    if _trace:
        _CACHE["last_results"] = res
    return out

